# revision 22
# baseline (speedup 1.0000x reference)
"""Trainium2 Bass kernel for nn_MultiHeadAttention_80187039416803.

Math (faithful to the reference, incl. the discarded-projection bug):
    q/k/v = split_heads(query/key/value)          # [B, H, S, dk]
    scores = q @ k^T / sqrt(dk)
    attn   = softmax(scores)
    ctx    = attn @ v                             # [B, H, S, dk]
    out    = merge_heads(ctx) @ w_concat.T        # [B, S, D]

Sharding: pure data-parallel over (batch, query-block).  8 cores each take
one (b, 512-query block) slice, compute full attention over all 16 heads for
their queries plus the final projection, and return a [512, 1024] slice of
the output.  No collectives needed.

Per-core kernel layout choices:
  - host pre-transposes q and k to [D, seq] ("feature-major") so the d_k=64
    contraction sits on the partition axis for the scores matmul;
  - softmax runs unnormalized (scores/8 ~ N(0,1): exp never overflows):
    exp on ScalarE straight out of PSUM, the row-sum Z comes for free from a
    ones-column prepended to v (host-built "vE" with 65 cols/head), and the
    1/Z scale is applied once on the tiny ctx^T [64, 512] tiles;
  - ctx^T tiles double as the lhsT of the final projection (no transposes
    anywhere on device);
  - PE instruction stream is software-pipelined: the scores matmuls for
    group g+1 are emitted before the attn@v matmuls of group g, so the PE
    never stalls behind the exp of the group it just produced.
All matmul inputs are bf16 (host-cast); accumulation is fp32 in PSUM.
"""

import numpy as np
import ml_dtypes
from contextlib import ExitStack

import concourse.bass as bass
import concourse.tile as tile
from concourse import bacc, mybir
from concourse.bass_utils import run_bass_kernel_spmd

BF16 = mybir.dt.bfloat16
F32 = mybir.dt.float32
NP_BF16 = ml_dtypes.bfloat16

B, S, D, H, DK = 2, 2048, 1024, 16, 64
Q = 512                 # queries per core
NCORES = 8
CH = S // 128           # 16 key chunks of 128
PAIRS = H // 2          # 8 head pairs
HD = DK + 1             # 65: 64 v columns + trailing ones column per head
GROUP = 3               # score chunks fused per ScalarE exp instruction

_CACHE = {}


def _build():
    nc = bacc.Bacc("TRN2", target_bir_lowering=False, debug=False)
    qT = nc.dram_tensor("qT", [D, Q], BF16, kind="ExternalInput").ap()
    kT = nc.dram_tensor("kT", [D, S], BF16, kind="ExternalInput").ap()
    vE = nc.dram_tensor("vE", [S, H * HD], BF16, kind="ExternalInput").ap()
    wT = nc.dram_tensor("wT", [D, D], BF16, kind="ExternalInput").ap()
    y = nc.dram_tensor("y", [Q, D], F32, kind="ExternalOutput").ap()

    with tile.TileContext(nc) as tc, ExitStack() as ctx:
        inp = ctx.enter_context(tc.tile_pool(name="inp", bufs=1))
        KT = [inp.tile([128, S], BF16, tag=f"kt{t}", name=f"kt{t}") for t in range(PAIRS)]
        QT = [inp.tile([128, Q], BF16, tag=f"qt{t}", name=f"qt{t}") for t in range(PAIRS)]
        VE = [inp.tile([128, H * HD], BF16, tag=f"ve{u}", name=f"ve{u}") for u in range(CH)]
        WT = [inp.tile([128, D], BF16, tag=f"wt{u}", name=f"wt{u}") for u in range(8)]
        XT = [inp.tile([128, Q], BF16, tag=f"xt{t}", name=f"xt{t}") for t in range(PAIRS)]

        # Loads roughly in consumption order: pair 0 needs KT/QT[0] + all VE.
        # KT/QT[0] split into column blocks so the first scores matmul can
        # start as soon as the first ~160KB lands instead of ~700KB.
        nc.sync.dma_start(QT[0][:], qT[0:128, :])
        for cb in range(4):
            nc.sync.dma_start(
                KT[0][:, cb * 512:(cb + 1) * 512],
                kT[0:128, cb * 512:(cb + 1) * 512],
            )
        for u in range(CH):
            nc.sync.dma_start(VE[u][:], vE[u * 128:(u + 1) * 128, :])
        for t in range(1, PAIRS):
            nc.sync.dma_start(KT[t][:], kT[t * 128:(t + 1) * 128, :])
            nc.sync.dma_start(QT[t][:], qT[t * 128:(t + 1) * 128, :])
        for u in range(8):
            nc.sync.dma_start(WT[u][:], wT[u * 128:(u + 1) * 128, :])

        # Warm the ScalarE exp table set during the initial DMA wait: the
        # first ACTIVATE triggers a ~2.7us ACT_TABLE_LOAD, so issue a tiny
        # dummy exp before any real work depends on it.
        warm = ctx.enter_context(tc.tile_pool(name="warm", bufs=1))
        wtile = warm.tile([1, 16], F32, name="warmt")
        nc.vector.memset(wtile[:], 0.0)
        wout = warm.tile([1, 16], F32, name="warmo")
        nc.scalar.activation(wout[:], wtile[:],
                             mybir.ActivationFunctionType.Exp, scale=1.0)

        # PSUM budget (8 banks): scores 2x3 + ctx/proj 2x1 = 8.
        spsum = ctx.enter_context(tc.tile_pool(name="spsum", bufs=2, space="PSUM"))
        cpsum = ctx.enter_context(tc.tile_pool(name="cpsum", bufs=2, space="PSUM"))

        # Warm the PE's HAM clock gate too: ~3.4us of sustained matmul
        # activity during the initial DMA wait lifts the PE from 1.2GHz to
        # 2.4GHz before the first real scores matmul issues.
        wlhs = warm.tile([1, 16], BF16, name="wlhs")
        nc.vector.memset(wlhs[:], 0.0)
        wrhs = warm.tile([1, 512], BF16, name="wrhs")
        nc.vector.memset(wrhs[:], 0.0)
        wps = cpsum.tile([16, 512], F32, tag="ctx", name="warmps")
        for _ in range(8):
            nc.tensor.matmul(wps[:], lhsT=wlhs[:], rhs=wrhs[:],
                             start=True, stop=True)
        epool = ctx.enter_context(tc.tile_pool(name="epool", bufs=3))
        evpool = ctx.enter_context(tc.tile_pool(name="evpool", bufs=3))
        rzpool = ctx.enter_context(tc.tile_pool(name="rzpool", bufs=2))
        reppool = ctx.enter_context(tc.tile_pool(name="reppool", bufs=2))

        # Flat list of score-chunk groups across all pairs.  Each slot is
        # (pair, head, key-chunk); heads interleave so consecutive K=64
        # matmuls hit different PE row-groups and run concurrently.
        groups = []
        for t in range(PAIRS):
            slots = [(t, h, j) for j in range(CH) for h in range(2)]
            for gi in range(0, len(slots), GROUP):
                groups.append(slots[gi:gi + GROUP])

        ctx_ps = {}     # (pair, head) -> psum tile accumulating ctx^T
        sp_of = {}      # group index -> (scores psum tile, e sbuf tile)

        def emit_mm1(g):
            cur = groups[g]
            t = cur[0][0]
            sp = spsum.tile([128, GROUP * Q], F32, tag="sp", name=f"sp{g}")
            for p, (t, h, j) in enumerate(cur):
                r = h * DK
                nc.tensor.matmul(
                    sp[:, p * Q:(p + 1) * Q],
                    lhsT=KT[t][r:r + DK, j * 128:(j + 1) * 128],
                    rhs=QT[t][r:r + DK, :],
                    start=True, stop=True,
                )
            e = epool.tile([128, GROUP * Q], BF16, tag="e", name=f"e{g}")
            glen = len(cur)
            nc.scalar.activation(
                e[:, 0:glen * Q], sp[:, 0:glen * Q],
                mybir.ActivationFunctionType.Exp, scale=0.125,
            )
            sp_of[g] = (sp, e)

        def emit_mm2(g):
            cur = groups[g]
            _, e = sp_of.pop(g)
            for p, (t, h, j) in enumerate(cur):
                if (t, h) not in ctx_ps:
                    ctx_ps[(t, h)] = cpsum.tile(
                        [HD, Q], F32, tag="ctx", name=f"ctx{t}_{h}"
                    )
                hh = 2 * t + h
                nc.tensor.matmul(
                    ctx_ps[(t, h)][:],
                    lhsT=VE[j][:, hh * HD:(hh + 1) * HD],
                    rhs=e[:, p * Q:(p + 1) * Q],
                    start=(j == 0), stop=(j == CH - 1),
                )
                if j == CH - 1 and h == 1:
                    emit_evict(t)

        def emit_evict(t):
            # Phase 1 for BOTH heads first: the cheap PSUM->SBUF copies free
            # the two ctx banks within ~2.5us so the next pair's attn@v isn't
            # stalled behind a full normalization chain on the DVE queue.
            cc, zr = {}, {}
            for h in range(2):
                cps = ctx_ps.pop((t, h))
                cc[h] = evpool.tile([HD, Q], F32, tag="ev", name=f"ev{t}_{h}")
                nc.vector.tensor_copy(cc[h][:], cps[:])
                # Plain copy moves Z to partition 0 (custom DVE ops only
                # handle partition-0-based APs) for the fast reciprocal.
                zr[h] = rzpool.tile([1, Q], F32, tag="zr", name=f"zr{t}_{h}")
                nc.vector.tensor_copy(zr[h][:], cps[DK:HD, :])
            for h in range(2):
                r = h * DK
                rz = rzpool.tile([1, Q], F32, tag="rz", name=f"rz{t}_{h}")
                nc.vector.reciprocal_approx_fast(rz[:], zr[h][:])
                rep = reppool.tile([HD, Q], F32, tag="rep", name=f"rep{t}_{h}")
                nc.gpsimd.partition_broadcast(rep[:], rz[:])
                nc.vector.tensor_mul(
                    XT[t][r:r + DK, :], cc[h][0:DK, :], rep[0:DK, :]
                )

        # Software pipeline: scores for group g+1 land in the PE stream
        # before attn@v for group g (which waits on the exp of group g).
        emit_mm1(0)
        for g in range(1, len(groups)):
            emit_mm1(g)
            emit_mm2(g - 1)
        emit_mm2(len(groups) - 1)

        # Final projection, tail-optimized: once the last exp has retired the
        # score-PSUM slots, 4 output tiles run their u=0..6 partial sums in
        # those slots (overlapping the last pair's eviction chain, which is
        # what the u=7 matmul and everything full-depth must wait for).
        outp = ctx.enter_context(tc.tile_pool(name="outp", bufs=3))
        accp = ctx.enter_context(tc.tile_pool(name="accp", bufs=4))
        PRE = [(0, 0), (0, 1), (1, 0), (1, 1)]
        partials = {}
        for qt, of in PRE:
            pq = spsum.tile([128, GROUP * Q], F32, tag="sp", name=f"pq{qt}_{of}")
            for u in range(7):
                nc.tensor.matmul(
                    pq[:, 0:512],
                    lhsT=XT[u][:, qt * 128:(qt + 1) * 128],
                    rhs=WT[u][:, of * 512:(of + 1) * 512],
                    start=(u == 0), stop=(u == 6),
                )
            acc = accp.tile([128, 512], F32, tag="acc", name=f"acc{qt}_{of}")
            nc.vector.tensor_copy(acc[:], pq[:, 0:512])
            partials[(qt, of)] = acc
        for qt, of in PRE:
            pp = cpsum.tile([128, 512], F32, tag="ctx", name=f"pp{qt}_{of}")
            nc.tensor.matmul(
                pp[:],
                lhsT=XT[7][:, qt * 128:(qt + 1) * 128],
                rhs=WT[7][:, of * 512:(of + 1) * 512],
                start=True, stop=True,
            )
            ot = outp.tile([128, 512], F32, tag="o", name=f"ot{qt}_{of}")
            nc.vector.tensor_add(ot[:], pp[:], partials[(qt, of)][:])
            nc.sync.dma_start(
                y[qt * 128:(qt + 1) * 128, of * 512:(of + 1) * 512], ot[:]
            )
        for qt in range(4):
            for of in range(2):
                if (qt, of) in partials:
                    continue
                pp = cpsum.tile([128, 512], F32, tag="ctx", name=f"pp{qt}_{of}")
                for u in range(8):
                    nc.tensor.matmul(
                        pp[:],
                        lhsT=XT[u][:, qt * 128:(qt + 1) * 128],
                        rhs=WT[u][:, of * 512:(of + 1) * 512],
                        start=(u == 0), stop=(u == 7),
                    )
                ot = outp.tile([128, 512], F32, tag="o", name=f"ot{qt}_{of}")
                nc.vector.tensor_copy(ot[:], pp[:])
                nc.sync.dma_start(
                    y[qt * 128:(qt + 1) * 128, of * 512:(of + 1) * 512], ot[:]
                )

    nc.compile()
    return nc


def _get_nc():
    if "nc" not in _CACHE:
        _CACHE["nc"] = _build()
    return _CACHE["nc"]


def _prep_in_maps(query, key, value, w_concat):
    query = np.asarray(query, dtype=np.float32)
    key = np.asarray(key, dtype=np.float32)
    value = np.asarray(value, dtype=np.float32)
    w_concat = np.asarray(w_concat, dtype=np.float32)

    wT = np.ascontiguousarray(w_concat.T).astype(NP_BF16)
    kT_b = [np.ascontiguousarray(key[b].T).astype(NP_BF16) for b in range(B)]
    vE_b = []
    for b in range(B):
        v3 = value[b].reshape(S, H, DK)
        ve = np.ones((S, H, HD), np.float32)
        ve[:, :, :DK] = v3
        vE_b.append(ve.reshape(S, H * HD).astype(NP_BF16))

    in_maps = []
    for c in range(NCORES):
        b, qi = divmod(c, NCORES // B)
        qs = qi * Q
        qT = np.ascontiguousarray(query[b, qs:qs + Q, :].T).astype(NP_BF16)
        in_maps.append({"qT": qT, "kT": kT_b[b], "vE": vE_b[b], "wT": wT})
    return in_maps


def _run(query, key, value, w_concat, **spmd_kwargs):
    nc = _get_nc()
    in_maps = _prep_in_maps(query, key, value, w_concat)
    res = run_bass_kernel_spmd(nc, in_maps, core_ids=list(range(NCORES)),
                               **spmd_kwargs)
    out = np.empty((B, S, D), np.float32)
    for c in range(NCORES):
        b, qi = divmod(c, NCORES // B)
        out[b, qi * Q:(qi + 1) * Q, :] = res.results[c]["y"]
    return out, res


def kernel(query, key, value, w_concat):
    out, _ = _run(query, key, value, w_concat)
    return out


# revision 23
# speedup vs baseline: 1.0019x; 1.0019x over previous
"""Trainium2 Bass kernel for nn_MultiHeadAttention_80187039416803.

Math (faithful to the reference, incl. the discarded-projection bug):
    q/k/v = split_heads(query/key/value)          # [B, H, S, dk]
    scores = q @ k^T / sqrt(dk)
    attn   = softmax(scores)
    ctx    = attn @ v                             # [B, H, S, dk]
    out    = merge_heads(ctx) @ w_concat.T        # [B, S, D]

Sharding: pure data-parallel over (batch, query-block).  8 cores each take
one (b, 512-query block) slice, compute full attention over all 16 heads for
their queries plus the final projection, and return a [512, 1024] slice of
the output.  No collectives needed.

Per-core kernel layout choices:
  - host pre-transposes q and k to [D, seq] ("feature-major") so the d_k=64
    contraction sits on the partition axis for the scores matmul;
  - softmax runs unnormalized (scores/8 ~ N(0,1): exp never overflows):
    exp on ScalarE straight out of PSUM, the row-sum Z comes for free from a
    ones-column prepended to v (host-built "vE" with 65 cols/head), and the
    1/Z scale is applied once on the tiny ctx^T [64, 512] tiles;
  - ctx^T tiles double as the lhsT of the final projection (no transposes
    anywhere on device);
  - PE instruction stream is software-pipelined: the scores matmuls for
    group g+1 are emitted before the attn@v matmuls of group g, so the PE
    never stalls behind the exp of the group it just produced.
All matmul inputs are bf16 (host-cast); accumulation is fp32 in PSUM.
"""

import numpy as np
import ml_dtypes
from contextlib import ExitStack

import concourse.bass as bass
import concourse.tile as tile
from concourse import bacc, mybir
from concourse.bass_utils import run_bass_kernel_spmd

BF16 = mybir.dt.bfloat16
F32 = mybir.dt.float32
NP_BF16 = ml_dtypes.bfloat16

B, S, D, H, DK = 2, 2048, 1024, 16, 64
Q = 512                 # queries per core
NCORES = 8
CH = S // 128           # 16 key chunks of 128
PAIRS = H // 2          # 8 head pairs
HD = DK + 1             # 65: 64 v columns + trailing ones column per head
GROUP = 3               # score chunks fused per ScalarE exp instruction

_CACHE = {}


def _build():
    nc = bacc.Bacc("TRN2", target_bir_lowering=False, debug=False)
    qT = nc.dram_tensor("qT", [D, Q], BF16, kind="ExternalInput").ap()
    kT = nc.dram_tensor("kT", [D, S], BF16, kind="ExternalInput").ap()
    vE = nc.dram_tensor("vE", [S, H * HD], BF16, kind="ExternalInput").ap()
    wT = nc.dram_tensor("wT", [D, D], BF16, kind="ExternalInput").ap()
    y = nc.dram_tensor("y", [Q, D], F32, kind="ExternalOutput").ap()

    with tile.TileContext(nc) as tc, ExitStack() as ctx:
        inp = ctx.enter_context(tc.tile_pool(name="inp", bufs=1))
        KT = [inp.tile([128, S], BF16, tag=f"kt{t}", name=f"kt{t}") for t in range(PAIRS)]
        QT = [inp.tile([128, Q], BF16, tag=f"qt{t}", name=f"qt{t}") for t in range(PAIRS)]
        VE = [inp.tile([128, H * HD], BF16, tag=f"ve{u}", name=f"ve{u}") for u in range(CH)]
        WT = [inp.tile([128, D], BF16, tag=f"wt{u}", name=f"wt{u}") for u in range(8)]
        XT = [inp.tile([128, Q], BF16, tag=f"xt{t}", name=f"xt{t}") for t in range(PAIRS)]

        # Loads roughly in consumption order: pair 0 needs KT/QT[0] + all VE.
        # KT/QT[0] split into column blocks so the first scores matmul can
        # start as soon as the first ~160KB lands instead of ~700KB.
        nc.sync.dma_start(QT[0][:], qT[0:128, :])
        for cb in range(4):
            nc.sync.dma_start(
                KT[0][:, cb * 512:(cb + 1) * 512],
                kT[0:128, cb * 512:(cb + 1) * 512],
            )
        for u in range(CH):
            nc.sync.dma_start(VE[u][:], vE[u * 128:(u + 1) * 128, :])
        for t in range(1, PAIRS):
            nc.sync.dma_start(KT[t][:], kT[t * 128:(t + 1) * 128, :])
            nc.sync.dma_start(QT[t][:], qT[t * 128:(t + 1) * 128, :])
        for u in range(8):
            nc.sync.dma_start(WT[u][:], wT[u * 128:(u + 1) * 128, :])

        # Warm the ScalarE exp table set during the initial DMA wait: the
        # first ACTIVATE triggers a ~2.7us ACT_TABLE_LOAD, so issue a tiny
        # dummy exp before any real work depends on it.
        warm = ctx.enter_context(tc.tile_pool(name="warm", bufs=1))
        wtile = warm.tile([1, 16], F32, name="warmt")
        nc.vector.memset(wtile[:], 0.0)
        wout = warm.tile([1, 16], F32, name="warmo")
        nc.scalar.activation(wout[:], wtile[:],
                             mybir.ActivationFunctionType.Exp, scale=1.0)

        # PSUM budget (8 banks): scores 2x3 + ctx/proj 2x1 = 8.
        spsum = ctx.enter_context(tc.tile_pool(name="spsum", bufs=2, space="PSUM"))
        cpsum = ctx.enter_context(tc.tile_pool(name="cpsum", bufs=2, space="PSUM"))
        epool = ctx.enter_context(tc.tile_pool(name="epool", bufs=3))
        evpool = ctx.enter_context(tc.tile_pool(name="evpool", bufs=3))
        rzpool = ctx.enter_context(tc.tile_pool(name="rzpool", bufs=2))
        reppool = ctx.enter_context(tc.tile_pool(name="reppool", bufs=2))

        # Flat CONTINUOUS list of score-chunk slots across all pairs, cut
        # into groups of 3; groups may straddle a pair boundary so the exp
        # cadence never hiccups (32 slots/pair is not divisible by 3).
        # Heads interleave so consecutive K=64 matmuls hit different PE
        # row-groups and run concurrently.
        slots = [(t, h, j) for t in range(PAIRS)
                 for j in range(CH) for h in range(2)]
        groups = [slots[gi:gi + GROUP] for gi in range(0, len(slots), GROUP)]

        ctx_ps = {}     # (pair, head) -> psum tile accumulating ctx^T
        sp_of = {}      # group index -> (scores psum tile, e sbuf tile)

        def emit_mm1(g):
            cur = groups[g]
            t = cur[0][0]
            sp = spsum.tile([128, GROUP * Q], F32, tag="sp", name=f"sp{g}")
            for p, (t, h, j) in enumerate(cur):
                r = h * DK
                nc.tensor.matmul(
                    sp[:, p * Q:(p + 1) * Q],
                    lhsT=KT[t][r:r + DK, j * 128:(j + 1) * 128],
                    rhs=QT[t][r:r + DK, :],
                    start=True, stop=True,
                )
            e = epool.tile([128, GROUP * Q], BF16, tag="e", name=f"e{g}")
            glen = len(cur)
            nc.scalar.activation(
                e[:, 0:glen * Q], sp[:, 0:glen * Q],
                mybir.ActivationFunctionType.Exp, scale=0.125,
            )
            sp_of[g] = (sp, e)

        def emit_mm2(g):
            cur = groups[g]
            _, e = sp_of.pop(g)
            for p, (t, h, j) in enumerate(cur):
                if (t, h) not in ctx_ps:
                    ctx_ps[(t, h)] = cpsum.tile(
                        [HD, Q], F32, tag="ctx", name=f"ctx{t}_{h}"
                    )
                hh = 2 * t + h
                nc.tensor.matmul(
                    ctx_ps[(t, h)][:],
                    lhsT=VE[j][:, hh * HD:(hh + 1) * HD],
                    rhs=e[:, p * Q:(p + 1) * Q],
                    start=(j == 0), stop=(j == CH - 1),
                )
                if j == CH - 1 and h == 1:
                    emit_evict(t)

        def emit_evict(t):
            # Phase 1 for BOTH heads first: the cheap PSUM->SBUF copies free
            # the two ctx banks within ~2.5us so the next pair's attn@v isn't
            # stalled behind a full normalization chain on the DVE queue.
            cc, zr = {}, {}
            for h in range(2):
                cps = ctx_ps.pop((t, h))
                cc[h] = evpool.tile([HD, Q], F32, tag="ev", name=f"ev{t}_{h}")
                nc.vector.tensor_copy(cc[h][:], cps[:])
                # Plain copy moves Z to partition 0 (custom DVE ops only
                # handle partition-0-based APs) for the fast reciprocal.
                zr[h] = rzpool.tile([1, Q], F32, tag="zr", name=f"zr{t}_{h}")
                nc.vector.tensor_copy(zr[h][:], cps[DK:HD, :])
            for h in range(2):
                r = h * DK
                rz = rzpool.tile([1, Q], F32, tag="rz", name=f"rz{t}_{h}")
                nc.vector.reciprocal_approx_fast(rz[:], zr[h][:])
                rep = reppool.tile([HD, Q], F32, tag="rep", name=f"rep{t}_{h}")
                nc.gpsimd.partition_broadcast(rep[:], rz[:])
                nc.vector.tensor_mul(
                    XT[t][r:r + DK, :], cc[h][0:DK, :], rep[0:DK, :]
                )

        # Software pipeline: scores for group g+1 land in the PE stream
        # before attn@v for group g (which waits on the exp of group g).
        emit_mm1(0)
        for g in range(1, len(groups)):
            emit_mm1(g)
            emit_mm2(g - 1)
        emit_mm2(len(groups) - 1)

        # Final projection, tail-optimized: once the last exp has retired the
        # score-PSUM slots, 4 output tiles run their u=0..6 partial sums in
        # those slots (overlapping the last pair's eviction chain, which is
        # what the u=7 matmul and everything full-depth must wait for).
        outp = ctx.enter_context(tc.tile_pool(name="outp", bufs=3))
        accp = ctx.enter_context(tc.tile_pool(name="accp", bufs=4))
        PRE = [(0, 0), (0, 1), (1, 0), (1, 1)]
        partials = {}
        for qt, of in PRE:
            pq = spsum.tile([128, GROUP * Q], F32, tag="sp", name=f"pq{qt}_{of}")
            for u in range(7):
                nc.tensor.matmul(
                    pq[:, 0:512],
                    lhsT=XT[u][:, qt * 128:(qt + 1) * 128],
                    rhs=WT[u][:, of * 512:(of + 1) * 512],
                    start=(u == 0), stop=(u == 6),
                )
            acc = accp.tile([128, 512], F32, tag="acc", name=f"acc{qt}_{of}")
            nc.vector.tensor_copy(acc[:], pq[:, 0:512])
            partials[(qt, of)] = acc
        for qt, of in PRE:
            pp = cpsum.tile([128, 512], F32, tag="ctx", name=f"pp{qt}_{of}")
            nc.tensor.matmul(
                pp[:],
                lhsT=XT[7][:, qt * 128:(qt + 1) * 128],
                rhs=WT[7][:, of * 512:(of + 1) * 512],
                start=True, stop=True,
            )
            ot = outp.tile([128, 512], F32, tag="o", name=f"ot{qt}_{of}")
            nc.vector.tensor_add(ot[:], pp[:], partials[(qt, of)][:])
            nc.sync.dma_start(
                y[qt * 128:(qt + 1) * 128, of * 512:(of + 1) * 512], ot[:]
            )
        for qt in range(4):
            for of in range(2):
                if (qt, of) in partials:
                    continue
                pp = cpsum.tile([128, 512], F32, tag="ctx", name=f"pp{qt}_{of}")
                for u in range(8):
                    nc.tensor.matmul(
                        pp[:],
                        lhsT=XT[u][:, qt * 128:(qt + 1) * 128],
                        rhs=WT[u][:, of * 512:(of + 1) * 512],
                        start=(u == 0), stop=(u == 7),
                    )
                ot = outp.tile([128, 512], F32, tag="o", name=f"ot{qt}_{of}")
                nc.vector.tensor_copy(ot[:], pp[:])
                nc.sync.dma_start(
                    y[qt * 128:(qt + 1) * 128, of * 512:(of + 1) * 512], ot[:]
                )

    nc.compile()
    return nc


def _get_nc():
    if "nc" not in _CACHE:
        _CACHE["nc"] = _build()
    return _CACHE["nc"]


def _prep_in_maps(query, key, value, w_concat):
    query = np.asarray(query, dtype=np.float32)
    key = np.asarray(key, dtype=np.float32)
    value = np.asarray(value, dtype=np.float32)
    w_concat = np.asarray(w_concat, dtype=np.float32)

    wT = np.ascontiguousarray(w_concat.T).astype(NP_BF16)
    kT_b = [np.ascontiguousarray(key[b].T).astype(NP_BF16) for b in range(B)]
    vE_b = []
    for b in range(B):
        v3 = value[b].reshape(S, H, DK)
        ve = np.ones((S, H, HD), np.float32)
        ve[:, :, :DK] = v3
        vE_b.append(ve.reshape(S, H * HD).astype(NP_BF16))

    in_maps = []
    for c in range(NCORES):
        b, qi = divmod(c, NCORES // B)
        qs = qi * Q
        qT = np.ascontiguousarray(query[b, qs:qs + Q, :].T).astype(NP_BF16)
        in_maps.append({"qT": qT, "kT": kT_b[b], "vE": vE_b[b], "wT": wT})
    return in_maps


def _run(query, key, value, w_concat, **spmd_kwargs):
    nc = _get_nc()
    in_maps = _prep_in_maps(query, key, value, w_concat)
    res = run_bass_kernel_spmd(nc, in_maps, core_ids=list(range(NCORES)),
                               **spmd_kwargs)
    out = np.empty((B, S, D), np.float32)
    for c in range(NCORES):
        b, qi = divmod(c, NCORES // B)
        out[b, qi * Q:(qi + 1) * Q, :] = res.results[c]["y"]
    return out, res


def kernel(query, key, value, w_concat):
    out, _ = _run(query, key, value, w_concat)
    return out


# revision 24
# speedup vs baseline: 1.0079x; 1.0061x over previous
"""Trainium2 Bass kernel for nn_MultiHeadAttention_80187039416803.

Math (faithful to the reference, incl. the discarded-projection bug):
    q/k/v = split_heads(query/key/value)          # [B, H, S, dk]
    scores = q @ k^T / sqrt(dk)
    attn   = softmax(scores)
    ctx    = attn @ v                             # [B, H, S, dk]
    out    = merge_heads(ctx) @ w_concat.T        # [B, S, D]

Sharding: pure data-parallel over (batch, query-block).  8 cores each take
one (b, 512-query block) slice, compute full attention over all 16 heads for
their queries plus the final projection, and return a [512, 1024] slice of
the output.  No collectives needed.

Per-core kernel layout choices:
  - host pre-transposes q and k to [D, seq] ("feature-major") so the d_k=64
    contraction sits on the partition axis for the scores matmul;
  - softmax runs unnormalized (scores/8 ~ N(0,1): exp never overflows):
    exp on ScalarE straight out of PSUM, the row-sum Z comes for free from a
    ones-column prepended to v (host-built "vE" with 65 cols/head), and the
    1/Z scale is applied once on the tiny ctx^T [64, 512] tiles;
  - ctx^T tiles double as the lhsT of the final projection (no transposes
    anywhere on device);
  - PE instruction stream is software-pipelined: the scores matmuls for
    group g+1 are emitted before the attn@v matmuls of group g, so the PE
    never stalls behind the exp of the group it just produced.
All matmul inputs are bf16 (host-cast); accumulation is fp32 in PSUM.
"""

import numpy as np
import ml_dtypes
from contextlib import ExitStack

import concourse.bass as bass
import concourse.tile as tile
from concourse import bacc, mybir
from concourse.bass_utils import run_bass_kernel_spmd

BF16 = mybir.dt.bfloat16
F32 = mybir.dt.float32
NP_BF16 = ml_dtypes.bfloat16

B, S, D, H, DK = 2, 2048, 1024, 16, 64
Q = 512                 # queries per core
NCORES = 8
CH = S // 128           # 16 key chunks of 128
PAIRS = H // 2          # 8 head pairs
HD = DK + 1             # 65: 64 v columns + trailing ones column per head
GROUP = 3               # score chunks fused per ScalarE exp instruction

_CACHE = {}


def _build():
    nc = bacc.Bacc("TRN2", target_bir_lowering=False, debug=False)
    qT = nc.dram_tensor("qT", [D, Q], BF16, kind="ExternalInput").ap()
    kT = nc.dram_tensor("kT", [D, S], BF16, kind="ExternalInput").ap()
    vE = nc.dram_tensor("vE", [S, H * HD], BF16, kind="ExternalInput").ap()
    wT = nc.dram_tensor("wT", [D, D], BF16, kind="ExternalInput").ap()
    y = nc.dram_tensor("y", [Q, D], F32, kind="ExternalOutput").ap()

    with tile.TileContext(nc) as tc, ExitStack() as ctx:
        inp = ctx.enter_context(tc.tile_pool(name="inp", bufs=1))
        KT = [inp.tile([128, S], BF16, tag=f"kt{t}", name=f"kt{t}") for t in range(PAIRS)]
        QT = [inp.tile([128, Q], BF16, tag=f"qt{t}", name=f"qt{t}") for t in range(PAIRS)]
        VE = [inp.tile([128, H * HD], BF16, tag=f"ve{u}", name=f"ve{u}") for u in range(CH)]
        WT = [inp.tile([128, D], BF16, tag=f"wt{u}", name=f"wt{u}") for u in range(8)]
        XT = [inp.tile([128, Q], BF16, tag=f"xt{t}", name=f"xt{t}") for t in range(PAIRS)]

        # Loads roughly in consumption order: pair 0 needs KT/QT[0] + all VE.
        # KT/QT[0] split into column blocks so the first scores matmul can
        # start as soon as the first ~160KB lands instead of ~700KB.
        nc.sync.dma_start(QT[0][:], qT[0:128, :])
        for cb in range(4):
            nc.sync.dma_start(
                KT[0][:, cb * 512:(cb + 1) * 512],
                kT[0:128, cb * 512:(cb + 1) * 512],
            )
        for u in range(CH):
            nc.sync.dma_start(VE[u][:], vE[u * 128:(u + 1) * 128, :])
        for t in range(1, PAIRS):
            nc.sync.dma_start(KT[t][:], kT[t * 128:(t + 1) * 128, :])
            nc.sync.dma_start(QT[t][:], qT[t * 128:(t + 1) * 128, :])
        for u in range(8):
            nc.sync.dma_start(WT[u][:], wT[u * 128:(u + 1) * 128, :])

        # Warm the ScalarE exp table set during the initial DMA wait: the
        # first ACTIVATE triggers a ~2.7us ACT_TABLE_LOAD, so issue a tiny
        # dummy exp before any real work depends on it.
        warm = ctx.enter_context(tc.tile_pool(name="warm", bufs=1))
        wtile = warm.tile([1, 16], F32, name="warmt")
        nc.vector.memset(wtile[:], 0.0)
        wout = warm.tile([1, 16], F32, name="warmo")
        nc.scalar.activation(wout[:], wtile[:],
                             mybir.ActivationFunctionType.Exp, scale=1.0)

        # PSUM budget (8 banks): scores 2x3 + ctx/proj 2x1 = 8.
        spsum = ctx.enter_context(tc.tile_pool(name="spsum", bufs=2, space="PSUM"))
        cpsum = ctx.enter_context(tc.tile_pool(name="cpsum", bufs=2, space="PSUM"))
        epool = ctx.enter_context(tc.tile_pool(name="epool", bufs=3))
        evpool = ctx.enter_context(tc.tile_pool(name="evpool", bufs=3))
        rzpool = ctx.enter_context(tc.tile_pool(name="rzpool", bufs=2))
        reppool = ctx.enter_context(tc.tile_pool(name="reppool", bufs=2))

        # Flat CONTINUOUS list of score-chunk slots across all pairs, cut
        # into groups of 3; groups may straddle a pair boundary so the exp
        # cadence never hiccups (32 slots/pair is not divisible by 3).
        # Heads interleave so consecutive K=64 matmuls hit different PE
        # row-groups and run concurrently.
        slots = [(t, h, j) for t in range(PAIRS)
                 for j in range(CH) for h in range(2)]
        groups = [slots[gi:gi + GROUP] for gi in range(0, len(slots), GROUP)]

        ctx_ps = {}     # (pair, head) -> psum tile accumulating ctx^T
        sp_of = {}      # group index -> (scores psum tile, e sbuf tile)

        def emit_mm1(g):
            cur = groups[g]
            t = cur[0][0]
            sp = spsum.tile([128, GROUP * Q], F32, tag="sp", name=f"sp{g}")
            for p, (t, h, j) in enumerate(cur):
                r = h * DK
                nc.tensor.matmul(
                    sp[:, p * Q:(p + 1) * Q],
                    lhsT=KT[t][r:r + DK, j * 128:(j + 1) * 128],
                    rhs=QT[t][r:r + DK, :],
                    start=True, stop=True,
                )
            e = epool.tile([128, GROUP * Q], BF16, tag="e", name=f"e{g}")
            glen = len(cur)
            nc.scalar.activation(
                e[:, 0:glen * Q], sp[:, 0:glen * Q],
                mybir.ActivationFunctionType.Exp, scale=0.125,
            )
            sp_of[g] = (sp, e)

        def emit_mm2(g):
            cur = groups[g]
            _, e = sp_of.pop(g)
            for p, (t, h, j) in enumerate(cur):
                if (t, h) not in ctx_ps:
                    ctx_ps[(t, h)] = cpsum.tile(
                        [HD, Q], F32, tag="ctx", name=f"ctx{t}_{h}"
                    )
                hh = 2 * t + h
                nc.tensor.matmul(
                    ctx_ps[(t, h)][:],
                    lhsT=VE[j][:, hh * HD:(hh + 1) * HD],
                    rhs=e[:, p * Q:(p + 1) * Q],
                    start=(j == 0), stop=(j == CH - 1),
                )
                if j == CH - 1 and h == 1:
                    emit_evict(t)

        def emit_evict(t):
            # Phase 1 for BOTH heads first: the cheap PSUM->SBUF copies free
            # the two ctx banks within ~2.5us so the next pair's attn@v isn't
            # stalled behind a full normalization chain on the DVE queue.
            # For the LAST pair nothing waits on the banks, so skip the big
            # copies and normalize straight out of PSUM — XT[7] (which gates
            # the projection tail) lands ~1us earlier.
            last = t == PAIRS - 1
            cc, zr = {}, {}
            for h in range(2):
                cps = ctx_ps.pop((t, h))
                if last:
                    cc[h] = cps
                else:
                    cc[h] = evpool.tile([HD, Q], F32, tag="ev", name=f"ev{t}_{h}")
                    nc.vector.tensor_copy(cc[h][:], cps[:])
                # Plain copy moves Z to partition 0 (custom DVE ops only
                # handle partition-0-based APs) for the fast reciprocal.
                zr[h] = rzpool.tile([1, Q], F32, tag="zr", name=f"zr{t}_{h}")
                nc.vector.tensor_copy(zr[h][:], cps[DK:HD, :])
            for h in range(2):
                r = h * DK
                rz = rzpool.tile([1, Q], F32, tag="rz", name=f"rz{t}_{h}")
                nc.vector.reciprocal_approx_fast(rz[:], zr[h][:])
                rep = reppool.tile([HD, Q], F32, tag="rep", name=f"rep{t}_{h}")
                nc.gpsimd.partition_broadcast(rep[:], rz[:])
                nc.vector.tensor_mul(
                    XT[t][r:r + DK, :], cc[h][0:DK, :], rep[0:DK, :]
                )

        # Software pipeline: scores for group g+1 land in the PE stream
        # before attn@v for group g (which waits on the exp of group g).
        emit_mm1(0)
        for g in range(1, len(groups)):
            emit_mm1(g)
            emit_mm2(g - 1)
        emit_mm2(len(groups) - 1)

        # Final projection, tail-optimized: once the last exp has retired the
        # score-PSUM slots, 4 output tiles run their u=0..6 partial sums in
        # those slots (overlapping the last pair's eviction chain, which is
        # what the u=7 matmul and everything full-depth must wait for).
        outp = ctx.enter_context(tc.tile_pool(name="outp", bufs=3))
        accp = ctx.enter_context(tc.tile_pool(name="accp", bufs=4))
        PRE = [(0, 0), (0, 1), (1, 0), (1, 1)]
        partials = {}
        for qt, of in PRE:
            pq = spsum.tile([128, GROUP * Q], F32, tag="sp", name=f"pq{qt}_{of}")
            for u in range(7):
                nc.tensor.matmul(
                    pq[:, 0:512],
                    lhsT=XT[u][:, qt * 128:(qt + 1) * 128],
                    rhs=WT[u][:, of * 512:(of + 1) * 512],
                    start=(u == 0), stop=(u == 6),
                )
            acc = accp.tile([128, 512], F32, tag="acc", name=f"acc{qt}_{of}")
            nc.vector.tensor_copy(acc[:], pq[:, 0:512])
            partials[(qt, of)] = acc
        for qt, of in PRE:
            pp = cpsum.tile([128, 512], F32, tag="ctx", name=f"pp{qt}_{of}")
            nc.tensor.matmul(
                pp[:],
                lhsT=XT[7][:, qt * 128:(qt + 1) * 128],
                rhs=WT[7][:, of * 512:(of + 1) * 512],
                start=True, stop=True,
            )
            ot = outp.tile([128, 512], F32, tag="o", name=f"ot{qt}_{of}")
            nc.vector.tensor_add(ot[:], pp[:], partials[(qt, of)][:])
            nc.sync.dma_start(
                y[qt * 128:(qt + 1) * 128, of * 512:(of + 1) * 512], ot[:]
            )
        for qt in range(4):
            for of in range(2):
                if (qt, of) in partials:
                    continue
                pp = cpsum.tile([128, 512], F32, tag="ctx", name=f"pp{qt}_{of}")
                for u in range(8):
                    nc.tensor.matmul(
                        pp[:],
                        lhsT=XT[u][:, qt * 128:(qt + 1) * 128],
                        rhs=WT[u][:, of * 512:(of + 1) * 512],
                        start=(u == 0), stop=(u == 7),
                    )
                ot = outp.tile([128, 512], F32, tag="o", name=f"ot{qt}_{of}")
                nc.vector.tensor_copy(ot[:], pp[:])
                nc.sync.dma_start(
                    y[qt * 128:(qt + 1) * 128, of * 512:(of + 1) * 512], ot[:]
                )

    nc.compile()
    return nc


def _get_nc():
    if "nc" not in _CACHE:
        _CACHE["nc"] = _build()
    return _CACHE["nc"]


def _prep_in_maps(query, key, value, w_concat):
    query = np.asarray(query, dtype=np.float32)
    key = np.asarray(key, dtype=np.float32)
    value = np.asarray(value, dtype=np.float32)
    w_concat = np.asarray(w_concat, dtype=np.float32)

    wT = np.ascontiguousarray(w_concat.T).astype(NP_BF16)
    kT_b = [np.ascontiguousarray(key[b].T).astype(NP_BF16) for b in range(B)]
    vE_b = []
    for b in range(B):
        v3 = value[b].reshape(S, H, DK)
        ve = np.ones((S, H, HD), np.float32)
        ve[:, :, :DK] = v3
        vE_b.append(ve.reshape(S, H * HD).astype(NP_BF16))

    in_maps = []
    for c in range(NCORES):
        b, qi = divmod(c, NCORES // B)
        qs = qi * Q
        qT = np.ascontiguousarray(query[b, qs:qs + Q, :].T).astype(NP_BF16)
        in_maps.append({"qT": qT, "kT": kT_b[b], "vE": vE_b[b], "wT": wT})
    return in_maps


def _run(query, key, value, w_concat, **spmd_kwargs):
    nc = _get_nc()
    in_maps = _prep_in_maps(query, key, value, w_concat)
    res = run_bass_kernel_spmd(nc, in_maps, core_ids=list(range(NCORES)),
                               **spmd_kwargs)
    out = np.empty((B, S, D), np.float32)
    for c in range(NCORES):
        b, qi = divmod(c, NCORES // B)
        out[b, qi * Q:(qi + 1) * Q, :] = res.results[c]["y"]
    return out, res


def kernel(query, key, value, w_concat):
    out, _ = _run(query, key, value, w_concat)
    return out
